# revision 23
# baseline (speedup 1.0000x reference)
"""Averaged Hausdorff loss kernel for Trainium2 (8 NeuronCores, SPMD).

Computes mean(min_j d(x_i, y_j)) + mean(min_i d(x_i, y_j)) for
set1 [8192, 256], set2 [8192, 256] using the Gram trick:
    d2[i,j] = ||x_i||^2 + ||y_j||^2 - 2 <x_i, y_j>

2D sharding: cores laid out 4x2. Core c = (a, b) with a = c // 2 (set1
quarter, 2048 rows) and b = c % 2 (set2 half, 4096 rows); each core
computes its [2048 x 4096] distance tile. Row-mins are partial over the
two b-halves, col-mins partial over the four a-quarters; both are
min-combined on host (cheap) before the final sqrt/means.

Per-core pipeline (q = ||.||^2/2, all mins on d2/2, sqrt'd on host):
    PE   : pg[j,i] = <x_i,y_j> - q1[i], via ONE fp8e4 DoubleRow matmul
           (K=256 in a single instruction) per 512-column span plus a
           K=1 fp16 aug matmul (ones x -q1row); augs of 4 consecutive
           spans sit at base partitions 0/32/64/96 so they pack into
           disjoint PE row-groups and run concurrently.
    ACT  : v = pg + (-q2[j]) bias, PSUM f32 -> SBUF fp16.
    DVE  : cacc[:,jt] = max_i v (tensor_reduce)
           racc = max(v, racc) elementwise (fp16 2x mode).
    colmin_d2 = -2*max(caccA,caccB); rowmin_d2 = -2*max_p racc.
"""

import os
import sys

import numpy as np

for _p in ("/opt/trn_rl_repo", os.path.expanduser("~/.axon_site/_ro/trn_rl_repo")):
    if os.path.isdir(_p) and _p not in sys.path:
        sys.path.insert(0, _p)

import concourse.bass as bass
import concourse.mybir as mybir
from concourse import bacc
from concourse.masks import make_identity
from concourse.tile import TileContext

N1 = 8192   # set1 rows
N2 = 8192   # set2 rows
D = 256     # feature dim
NCORES = 8
ADIV = 4    # set1 split
BDIV = 2    # set2 split
S1L = N1 // ADIV   # 2048 set1 rows per core
S2L = N2 // BDIV   # 4096 set2 rows per core
P = 128
RT = S1L // P      # 16 i-tiles per core
JT = S2L // P      # 32 j-tiles per core
KT = D // P        # 2 contraction chunks
IH = S1L // 1024   # 2 i-spans of 1024 per psum tile row
F32 = mybir.dt.float32
F16 = mybir.dt.float16
FP8 = mybir.dt.float8e4
NEG = -60000.0
HALF_SQRT = 0.70710677  # sqrt(0.5); Square(x*s) = x^2/2

A = mybir.AluOpType
AX = mybir.AxisListType
AF = mybir.ActivationFunctionType
PM = mybir.MatmulPerfMode

GRP = 4  # natural 128-row tiles converted/transposed per batch


def build_kernel(use_doublerow=True):
    nc = bacc.Bacc()
    s1 = nc.declare_dram_parameter("s1", [S1L, D], F32, isOutput=False)
    s2 = nc.declare_dram_parameter("s2", [S2L, D], F32, isOutput=False)
    rowmin = nc.declare_dram_parameter("rowmin", [P, RT], F32, isOutput=True)
    colmin = nc.declare_dram_parameter("colmin", [P, JT], F32, isOutput=True)

    with TileContext(nc) as tc:
        with (
            tc.tile_pool(name="persist", bufs=1) as persist,
            tc.tile_pool(name="nat32", bufs=16) as nat32p,   # f32 natural tiles
            tc.tile_pool(name="nat16", bufs=3) as nat16p,    # fp16 natural (pre-transpose)
            tc.tile_pool(name="vpool", bufs=2) as vpool,
            tc.tile_pool(name="junk", bufs=2) as junkp,
        ):
            # ---- persistent SBUF tensors ----
            s2T8 = persist.tile([P, KT, S2L], FP8)
            s1T8 = persist.tile([P, KT, S1L], FP8)
            nq2 = persist.tile([P, JT], F32)        # -||y||^2/2 per j (ACT bias)
            q2pos = persist.tile([P, JT], F32)      # +q2 staging for ACT-accum tiles
            negq1rep = persist.tile([P, S1L], F16)  # -q1 row at partitions 0/32/64/96
            onesrep = persist.tile([P, P], F16)     # ones rows (aug lhsT)
            racc = [persist.tile([P, S1L], F16, name=f"racc{k}", tag=f"racc{k}") for k in range(2)]
            cacc = persist.tile([P, IH, JT], F32)   # per-i-span col maxes
            ident_h = persist.tile([P, P], F16)
            neghalf = persist.tile([P, 1], F16)
            consts_f = persist.tile([P, 2], F32)  # cols: -0.5, 1.0
            s1sq = persist.tile([P, KT, S1L], F16)
            m1 = persist.tile([P, RT], F32)
            rfix = persist.tile([P, RT], F32)
            cfix = persist.tile([P, JT], F32)

            make_identity(nc, ident_h)
            nc.gpsimd.memset(consts_f[:, 0:1], -0.5)
            nc.gpsimd.memset(consts_f[:, 1:2], 1.0)
            nc.gpsimd.memset(racc[0], NEG)
            nc.gpsimd.memset(onesrep, 1.0)
            nc.vector.tensor_copy(neghalf, consts_f[:, 0:1])

            def load_convert_transpose(src, t0, ntiles, dstT, psA, q_ops):
                """DMA f32 tiles [t0, t0+ntiles), convert to fp16, PE-transpose
                per k-chunk into one fp16 PSUM tile, evacuate as fp8 into
                dstT[:, k, t0*P : (t0+ntiles)*P]."""
                nat32 = nat32p.tile([P, ntiles * D], F32, tag="nat32")
                for b in range(ntiles):
                    t = t0 + b
                    nc.sync.dma_start(
                        nat32[:, b * D:(b + 1) * D], src[t * P:(t + 1) * P, :]
                    )
                nat16 = nat16p.tile([P, ntiles * D], F16, tag="nat16")
                nc.vector.tensor_copy(nat16, nat32)
                for b in range(ntiles):
                    q_ops(t0 + b, nat32[:, b * D:(b + 1) * D])
                pt = psA.tile([P, KT, ntiles * P], F16, tag="ptrans")
                for k in range(KT):
                    for b in range(ntiles):
                        nc.tensor.transpose(
                            pt[:, k, b * P:(b + 1) * P],
                            nat16[:, b * D + k * P: b * D + (k + 1) * P],
                            ident_h,
                        )
                dst = dstT[:, :, t0 * P:(t0 + ntiles) * P]
                if (t0 // ntiles) % 2 == 0:
                    nc.scalar.copy(dst, pt)
                else:
                    nc.vector.tensor_copy(dst, pt)

            def q2_ops(t, nat):
                junk = junkp.tile([P, D], F32, tag="q2junk")
                if t % 2 == 0:
                    # ACT: junk = (x*sqrt(.5))^2, accum = +q2 (negated later)
                    nc.scalar.activation(
                        junk, nat, AF.Square, scale=HALF_SQRT,
                        accum_out=q2pos[:, t:t + 1],
                    )
                else:
                    # DVE: junk = (x * -0.5) * x, accum = -q2 directly
                    nc.vector.scalar_tensor_tensor(
                        out=junk, in0=nat, scalar=-0.5, in1=nat,
                        op0=A.mult, op1=A.mult,
                        accum_out=nq2[:, t:t + 1],
                    )

            def no_q(t, nat):
                pass

            with tc.tile_pool(name="psA", bufs=2, space="PSUM") as psA:
                # ---- s1 first (stage B needs all of it) ----
                for g in range(RT // GRP):
                    load_convert_transpose(s1, g * GRP, GRP, s1T8, psA, no_q)

                # negq1row: -q1 via neghalf^T @ square(s1T8); replicated to
                # partitions 0/32/64/96 of negq1rep for packed aug matmuls.
                nc.scalar.square(s1sq, s1T8)
                with tc.tile_pool(name="psP", bufs=2, space="PSUM") as psP:
                    for blk in range(S1L // 512):
                        pr = psP.tile([1, 512], F32, tag="prep")
                        for k in range(KT):
                            nc.tensor.matmul(
                                pr,
                                lhsT=neghalf,
                                rhs=s1sq[:, k, blk * 512:(blk + 1) * 512],
                                start=(k == 0), stop=(k == KT - 1),
                            )
                        for rep in range(4):
                            nc.vector.tensor_copy(
                                negq1rep[32 * rep:32 * rep + 1, blk * 512:(blk + 1) * 512],
                                pr,
                            )

                # ---- s2 groups + interleaved stage B ----
                with tc.tile_pool(name="psB", bufs=3, space="PSUM") as psB:
                    HALF = 512

                    def stage_b(jt):
                        """One j-tile: IH psum tiles of [P, 1024] over the
                        2048 i's; evac to one [P, S1L] v; reduce + racc."""
                        pgs = []
                        for ih in range(IH):
                            pg = psB.tile([P, 1024], F32, tag="gram")
                            pgs.append(pg)
                            for h in range(2):
                                sl0 = ih * 1024 + h * HALF
                                sl = slice(sl0, sl0 + HALF)
                                psl = slice(h * HALF, (h + 1) * HALF)
                                if use_doublerow:
                                    nc.tensor.matmul(
                                        pg[:, psl],
                                        lhsT=s2T8[:, :, jt * P:(jt + 1) * P],
                                        rhs=s1T8[:, :, sl],
                                        start=True, stop=False,
                                        perf_mode=PM.DoubleRow,
                                    )
                                else:
                                    for k in range(KT):
                                        nc.tensor.matmul(
                                            pg[:, psl],
                                            lhsT=s2T8[:, k, jt * P:(jt + 1) * P],
                                            rhs=s1T8[:, k, sl],
                                            start=(k == 0), stop=False,
                                        )
                        # packed augs: 4 (ih, half) slots at partitions 0/32/64/96
                        slot = 0
                        for ih in range(IH):
                            pg = pgs[ih]
                            for h in range(2):
                                sl0 = ih * 1024 + h * HALF
                                sl = slice(sl0, sl0 + HALF)
                                psl = slice(h * HALF, (h + 1) * HALF)
                                bp = 32 * slot
                                nc.tensor.matmul(
                                    pg[:, psl],
                                    lhsT=onesrep[bp:bp + 1, :],
                                    rhs=negq1rep[bp:bp + 1, sl],
                                    start=False, stop=True,
                                    tile_position=(bp, 0),
                                )
                                slot += 1
                        # ACT evac + bias into one [P, S1L] v, then DVE
                        v = vpool.tile([P, S1L], F16, tag="v")
                        for ih in range(IH):
                            nc.scalar.activation(
                                v[:, ih * 1024:(ih + 1) * 1024], pgs[ih],
                                AF.Identity, bias=nq2[:, jt:jt + 1],
                            )
                            nc.vector.tensor_reduce(
                                cacc[:, ih, jt:jt + 1],
                                v[:, ih * 1024:(ih + 1) * 1024],
                                axis=AX.X, op=A.max,
                            )
                        nc.vector.tensor_tensor(
                            racc[(jt + 1) % 2], v, racc[jt % 2], A.max
                        )

                    for g in range(JT // GRP):
                        load_convert_transpose(s2, g * GRP, GRP, s2T8, psA, q2_ops)
                        # negate the ACT-accumulated (even) q2 columns
                        nc.vector.tensor_scalar_mul(
                            nq2[:, g * GRP:(g + 1) * GRP:2],
                            q2pos[:, g * GRP:(g + 1) * GRP:2],
                            -1.0,
                        )
                        for jt in range(g * GRP, (g + 1) * GRP):
                            stage_b(jt)

            # ---- finalize ----
            with tc.tile_pool(name="psC", bufs=2, space="PSUM") as psC:
                rfin = racc[JT % 2]
                for b in range(RT):
                    ptc = psC.tile([P, P], F16, tag="ptc")
                    nc.tensor.transpose(ptc, rfin[:, b * P:(b + 1) * P], ident_h)
                    nc.vector.tensor_reduce(
                        m1[:, b:b + 1], ptc, axis=AX.X, op=A.max
                    )
                # rowmin_d2 = -2*m1; colmin_d2 = -2*max_ih(cacc)
                nc.vector.tensor_scalar_mul(rfix, m1, -2.0)
                nc.vector.tensor_tensor(cfix, cacc[:, 0, :], cacc[:, 1, :], A.max)
                nc.vector.tensor_scalar_mul(cfix, cfix, -2.0)
                nc.sync.dma_start(rowmin[:, :], rfix)
                nc.sync.dma_start(colmin[:, :], cfix)

    nc.compile()
    return nc


_CACHE: dict = {}


def _built(key=("dr",)):
    if key not in _CACHE:
        _CACHE[key] = build_kernel(use_doublerow="dr" in key)
    return _CACHE[key]


def run_on_cores(set1: np.ndarray, set2: np.ndarray, variant=("dr",), **kw):
    """Run the SPMD kernel; returns (rowmin_d2 [8192], colmin_d2 [8192], results)."""
    from concourse.bass_utils import run_bass_kernel_spmd

    nc = _built(tuple(variant))
    in_maps = [
        {
            "s1": np.ascontiguousarray(set1[(c // BDIV) * S1L:(c // BDIV + 1) * S1L]),
            "s2": np.ascontiguousarray(set2[(c % BDIV) * S2L:(c % BDIV + 1) * S2L]),
        }
        for c in range(NCORES)
    ]
    res = run_bass_kernel_spmd(nc, in_maps, core_ids=list(range(NCORES)), **kw)
    # rowmin: core (a,b) covers i quarter a (partial over j half b)
    # colmin: core (a,b) covers j half b (partial over i quarter a)
    row_q = []
    for a in range(ADIV):
        parts = [res.results[a * BDIV + b]["rowmin"].T.reshape(-1) for b in range(BDIV)]
        row_q.append(np.min(np.stack(parts), axis=0))
    rowmin_d2 = np.concatenate(row_q)                # [8192], by global row
    col_h = []
    for b in range(BDIV):
        parts = [res.results[a * BDIV + b]["colmin"].T.reshape(-1) for a in range(ADIV)]
        col_h.append(np.min(np.stack(parts), axis=0))
    colmin_d2 = np.concatenate(col_h)                # [8192], by global col
    return rowmin_d2, colmin_d2, res


def kernel(set1, set2) -> np.ndarray:
    set1 = np.asarray(set1, dtype=np.float32)
    set2 = np.asarray(set2, dtype=np.float32)
    rowmin_d2, colmin_d2, _ = run_on_cores(set1, set2)
    t1 = np.sqrt(np.maximum(rowmin_d2, 0.0), dtype=np.float32).mean(dtype=np.float32)
    t2 = np.sqrt(np.maximum(colmin_d2, 0.0), dtype=np.float32).mean(dtype=np.float32)
    return np.array(np.float32(t1) + np.float32(t2), dtype=np.float32)
